# revision 36
# baseline (speedup 1.0000x reference)
"""Trainium2 Bass kernel for the DiCNN (WaveNet-like) module.

Sharding: pure data parallelism - 4 batch items per core on 8 cores.
On-chip layout: channels on partitions, time on the free dim; the four
batch items are stacked as 4x32-partition bands (block-diag weights),
2x64 bands for the 64-channel causal layer.

v2 design notes (vs the 151us baseline):
- y is stored bf16 (halves the dominant output-store DMA stream) and
  upcast to fp32 host-side. rel tolerance is 2e-2; bf16 adds ~4e-3.
- skip path accumulates in PSUM: ws0@g0 (start) and ws1@g1 (stop)
  share one bank, so the skip copy + one STT disappear.
- s1 is kept in pair layout [97, T]: bands 2q at partitions 0:33 and
  2q+1 at 64:97 (ones rows at 32/96 fold the w_sk2 bias). One relu op
  materializes both bands, and the final-conv stationaries alternate
  PE row strips 0/64 so the HW pulls LDWEIGHTS ahead of in-flight
  matmuls (no row-group conflict) - hiding 16 weight loads per tile.
- final-conv time chunks are stride-4 interleaved (chunk j covers
  t = t0+4p+j), so the 4 copied chunks land t-major in SBUF and one
  DMA per (band, tile) stores 512 timesteps contiguously.
- gate multiplies run on GpSimd (only SBUF-resident op available to
  it; PSUM is V/S-only), PSUM->SBUF out-copies split scalar/vector.
- causal conv uses [128, 64] col-split stationaries (cheaper LDW).
"""

import numpy as np
import ml_dtypes

import concourse.bacc as bacc
import concourse.tile as tile
from concourse import mybir
from concourse.bass_utils import run_bass_kernel_spmd

BF16 = mybir.dt.bfloat16
FP32 = mybir.dt.float32

B, T, C_IN, HID, C_OUT, K = 32, 4096, 64, 32, 448, 2
N_CORES = 8
BPC = B // N_CORES          # batches per core = 4
TT = 512                    # time-tile size
NT = T // TT                # 8 tiles
NC4 = TT // 4               # 128 columns per interleaved chunk
XROWS = 4112                # 4097 rounded up (row 0 = causal zero pad)
XSPLIT = 2564               # x loads split so tiles 0-4 unblock early
N_WARMUP = 14               # dependency-free warm-up matmuls

AF = mybir.ActivationFunctionType
ALU = mybir.AluOpType

# which of the 16 out-copies per tile run on the scalar engine
# (indices 4*j+b; spread through the phase so pout slots recycle
# steadily and neither engine's queue bunches up)
SCALAR_COPY = {1, 3, 4, 6, 12, 14}

_cached_nc = None


def _f(x):
    return np.asarray(x, dtype=np.float32)


def _bf(x):
    return np.asarray(x, dtype=np.float32).astype(ml_dtypes.bfloat16)


def _tile4(v):
    return np.tile(_f(v).reshape(-1), 4)


def prepare_weights(w_causal, b_causal, wd0, bd0, ws0, bs0, wo0, bo0,
                    wd1, bd1, ws1, bs1, wo1, bo1, w_sk1, b_sk1, w_sk2, b_sk2):
    """Host-side weight layout transforms (identical for every core)."""
    del wo1, bo1  # dead code: z after the last block is never used

    def diag4(w32):
        s = np.zeros((128, 128), np.float32)
        for i in range(4):
            s[32 * i:32 * i + 32, 32 * i:32 * i + 32] = w32
        return s

    wc = np.zeros((128, 4, 128), np.float32)
    for p in range(2):
        for k in range(2):
            wcT = _f(w_causal)[:, :, k].T
            s = np.zeros((128, 128), np.float32)
            s[0:64, 64 * p:64 * p + 32] = wcT
            s[64:128, 64 * p + 32:64 * p + 64] = wcT
            wc[:, 2 * p + k, :] = s

    wd = np.zeros((128, 4, 128), np.float32)
    for blk, w in enumerate((wd0, wd1)):
        for k in range(2):
            wd[:, 2 * blk + k, :] = diag4(_f(w)[:, :, k].T)

    wsr = np.zeros((128, 2, 128), np.float32)
    wsr[:, 0, :] = diag4(_f(ws0)[:, :, 0].T)
    wsr[:, 1, :] = diag4(_f(wo0)[:, :, 0].T)
    ws1d = diag4(_f(ws1)[:, :, 0].T)

    # w_sk1 pair stationaries: wsk1[:, q, :] covers batch bands 2q, 2q+1;
    # output cols 32 and 96 stay 0 so relu(0 + 1.0 bias) makes ones rows.
    wsk1 = np.zeros((128, 2, 97), np.float32)
    w1T = _f(w_sk1)[:, :, 0].T
    for q in range(2):
        wsk1[64 * q:64 * q + 32, q, 0:32] = w1T
        wsk1[64 * q + 32:64 * q + 64, q, 64:96] = w1T

    # final conv moving operand, duplicated at partition bases 0 and 64
    w2d = np.zeros((97, 448), np.float32)
    for base in (0, 64):
        w2d[base:base + 32, :] = _f(w_sk2)[:, :, 0].T
        w2d[base + 32, :] = _f(b_sk2)

    bvecs = np.zeros((128, 6), np.float32)
    bvecs[:, 0] = _tile4(b_causal)
    bvecs[:, 1] = _tile4(bd0)
    bvecs[:, 2] = _tile4(bd1)
    bvecs[:, 3] = _tile4(bo0)
    bvecs[:, 4] = _tile4(_f(bs0) + _f(bs1))
    bvecs[0:32, 5] = _f(b_sk1)
    bvecs[32, 5] = 1.0
    bvecs[64:96, 5] = _f(b_sk1)
    bvecs[96, 5] = 1.0

    return dict(
        wc=_bf(wc), wd=_bf(wd), wsr=_bf(wsr), ws1d=_bf(ws1d),
        wsk1=_bf(wsk1), w2d=_bf(w2d), bvecs=np.ascontiguousarray(bvecs),
    )


def prepare_x(x, core):
    """Per-core channel-major input staging array [2, 128, XROWS] bf16.

    Column 0 is the causal zero pad (t=-1); column 1+t holds x[b, t, :]
    for the two batches of pair p stacked on the partition axis.
    """
    xT = np.zeros((2, 128, XROWS), ml_dtypes.bfloat16)
    xb = _bf(x)
    for p in range(2):
        xT[p, 0:64, 1:1 + T] = xb[4 * core + 2 * p].T
        xT[p, 64:128, 1:1 + T] = xb[4 * core + 2 * p + 1].T
    return xT


def build_nc():
    nc = bacc.Bacc("TRN2", target_bir_lowering=False, debug=False,
                   num_devices=N_CORES)

    xT_d = nc.dram_tensor("xT", [2, 128, XROWS], BF16, kind="ExternalInput")
    wc_d = nc.dram_tensor("wc", [128, 4, 128], BF16, kind="ExternalInput")
    wd_d = nc.dram_tensor("wd", [128, 4, 128], BF16, kind="ExternalInput")
    wsr_d = nc.dram_tensor("wsr", [128, 2, 128], BF16, kind="ExternalInput")
    ws1_d = nc.dram_tensor("ws1d", [128, 128], BF16, kind="ExternalInput")
    wsk1_d = nc.dram_tensor("wsk1", [128, 2, 97], BF16, kind="ExternalInput")
    w2_d = nc.dram_tensor("w2d", [97, 448], BF16, kind="ExternalInput")
    bv_d = nc.dram_tensor("bvecs", [128, 6], FP32, kind="ExternalInput")
    # [b, it, p, j, co] has the same linear layout as [b, t, co] with
    # t = 512*it + 4*p + j  (the stride-4 chunk interleave).
    y_d = nc.dram_tensor("y", [BPC, NT, NC4, 4, C_OUT], BF16,
                         kind="ExternalOutput")

    with tile.TileContext(nc) as tc:
        with (
            tc.tile_pool(name="const", bufs=1) as const,
            tc.tile_pool(name="persist", bufs=1) as persist,
            tc.tile_pool(name="act", bufs=4) as actp,
            tc.tile_pool(name="gtile", bufs=6) as gtp,
            tc.tile_pool(name="outsb", bufs=6) as outsbp,
            tc.tile_pool(name="pbody", bufs=4, space="PSUM") as pbp,
            tc.tile_pool(name="pout", bufs=4, space="PSUM") as poutp,
        ):
            # ---- load order matters: the sync HWDGE ring drains FIFO.
            # Tiny wc+bvecs first (warm-up + ACT preload), then the early
            # x halves, then the other weights, then the late x halves.
            x_s = [persist.tile([128, XROWS], BF16, tag=f"x{p}", name=f"x_s{p}")
                   for p in range(2)]
            wc_s = const.tile([128, 4, 128], BF16)
            nc.sync.dma_start(wc_s[:], wc_d.ap())
            bv_s = const.tile([128, 6], FP32)
            nc.sync.dma_start(bv_s[:], bv_d.ap())
            for p in range(2):
                nc.sync.dma_start(x_s[p][:, 0:XSPLIT], xT_d[p][:, 0:XSPLIT])
            wd_s = const.tile([128, 4, 128], BF16)
            nc.sync.dma_start(wd_s[:], wd_d.ap())
            wsr_s = const.tile([128, 2, 128], BF16)
            nc.sync.dma_start(wsr_s[:], wsr_d.ap())
            ws1_s = const.tile([128, 128], BF16)
            nc.sync.dma_start(ws1_s[:], ws1_d.ap())
            wsk1_s = const.tile([128, 2, 97], BF16)
            nc.sync.dma_start(wsk1_s[:], wsk1_d.ap())
            w2_s = const.tile([97, 448], BF16)
            nc.sync.dma_start(w2_s[:], w2_d.ap())
            for p in range(2):
                nc.sync.dma_start(x_s[p][:, XSPLIT:XROWS],
                                  xT_d[p][:, XSPLIT:XROWS])

            # preload every ACT function table set during the x transfer
            # (the first real activation would otherwise stall ~1.3us)
            scratch = const.tile([128, 8], FP32)
            nc.scalar.activation(scratch[:, 0:1], bv_s[:, 0:1], AF.Tanh)
            nc.scalar.activation(scratch[:, 1:2], bv_s[:, 0:1], AF.Sigmoid)
            nc.scalar.activation(scratch[:, 2:3], bv_s[:, 0:1], AF.Relu)
            nc.scalar.add(scratch[:, 3:4], bv_s[:, 0:1], 0.0)
            nc.scalar.copy(scratch[:, 4:5], bv_s[:, 0:1])

            bcausal = bv_s[:, 0:1]
            bd_v = (bv_s[:, 1:2], bv_s[:, 2:3])
            bo0_v = bv_s[:, 3:4]
            bskip_v = bv_s[:, 4:5]
            bsk1_v = bv_s[:, 5:6]

            # ---- persistent activations ----
            z0_s = persist.tile([128, 4100], BF16, tag="z0")
            nc.vector.memset(z0_s[:, 0:1], 0.0)
            z1_s = persist.tile([128, 4100], BF16, tag="z1")
            nc.vector.memset(z1_s[:, 0:2], 0.0)
            # s1 pair buffers: [97, T/4, 4]; band 2q at partitions 0:33,
            # band 2q+1 at 64:97 (ones rows at 32 / 96).
            s1_s = [persist.tile([97, T // 4, 4], BF16, tag=f"s1_{q}",
                                 name=f"s1_{q}") for q in range(2)]

            # ---- PE warm-up burst (overlaps the x loads) ----
            wu_t = persist.tile([128, TT], BF16, tag="wu")
            nc.vector.memset(wu_t[:], 0.0)
            hb_cnt = [0]

            def heartbeat(n):
                """Dependency-free PE filler matmuls: keep the HAM activity
                window busy across dependency stalls so the 2.4 GHz clock
                state is never lost."""
                for _ in range(n):
                    pwu = poutp.tile([128, C_OUT], FP32, tag="po",
                                     name=f"pwu_{hb_cnt[0]}")
                    hb_cnt[0] += 1
                    nc.tensor.matmul(pwu[:], wc_s[:, 0, :], wu_t[:, 0:C_OUT],
                                     start=True, stop=True)

            heartbeat(N_WARMUP)

            def filler(tile_ap, lhsT):
                """Garbage matmul into a pool slot that is free at this
                point; the real producer overwrites it with start=True.
                Pads PE idle gaps so the HAM activity window stays busy."""
                hb_cnt[0] += 1
                nc.tensor.matmul(tile_ap, lhsT, wu_t[:],
                                 start=True, stop=True)

            outsb_tiles = {}
            g0s, g1s, s0s, psAs, pzs = {}, {}, {}, {}, {}

            def outsb_of(it, b):
                key = (it, b)
                if key not in outsb_tiles:
                    outsb_tiles[key] = outsbp.tile(
                        [128, 4, C_OUT], BF16, tag="o", name=f"o_{it}_{b}")
                return outsb_tiles[key]

            def emit_out_chunk(it, j, dmas=False):
                """Final-conv work for (tile it, interleaved chunk j):
                4 matmuls (row strips alternate 0/64 so LDW pulls ahead)
                + 4 PSUM->SBUF bf16 copies. Chunk j covers t = t0+4p+j."""
                if it < 0:
                    heartbeat(3)
                    return
                c0 = it * NC4
                for b in range(4):
                    q, hi = b // 2, b % 2
                    base = 64 * hi
                    po = poutp.tile([128, C_OUT], FP32, tag="po",
                                    name=f"po_{it}_{j}_{b}")
                    nc.tensor.matmul(
                        po[:], s1_s[q][base:base + 33, c0:c0 + NC4, j],
                        w2_s[base:base + 33, :], start=True, stop=True)
                    o_t = outsb_of(it, b)
                    if (4 * j + b) in SCALAR_COPY:
                        nc.scalar.copy(o_t[:, j, :], po[:])
                    else:
                        nc.vector.tensor_copy(o_t[:, j, :], po[:])
                if dmas:
                    for b in range(4):
                        nc.sync.dma_start(y_d[b, it], outsb_of(it, b)[:])

            def emit_phase(t):
                """Software-pipelined phase: stage1 of tile t, stage2+3 of
                t-1, final conv + store of t-2. All cross-stage producers
                complete >=1 phase (or several us) before their consumers,
                so no engine queue blocks long."""
                t1, t2, t3 = t, t - 1, t - 2

                # -- causal conv(t1): 2 batch-pair groups of 2 taps -> z0
                if 0 <= t1 < NT:
                    t0 = TT * t1
                    pz = pzs.pop(t1, None)
                    if pz is None:
                        pz = pbp.tile([128, TT], FP32, tag="ps",
                                      name=f"pz_{t1}")
                    for p in range(2):
                        rhs = (x_s[p][:, t0:t0 + TT],
                               x_s[p][:, t0 + 1:t0 + 1 + TT])
                        for k in range(2):
                            nc.tensor.matmul(
                                pz[64 * p:64 * p + 64, :],
                                wc_s[:, 2 * p + k, 64 * p:64 * p + 64], rhs[k],
                                start=(k == 0), stop=(k == 1))
                    nc.scalar.add(z0_s[:, 1 + t0:1 + t0 + TT], pz[:], bcausal)

                # -- skip0/res0 matmuls (t2): consume g0 from last phase
                if 0 <= t2 < NT:
                    t0 = TT * t2
                    psA = pbp.tile([128, TT], FP32, tag="ps",
                                   name=f"psA_{t2}")
                    psAs[t2] = psA
                    nc.tensor.matmul(psA[:], wsr_s[:, 0, :], g0s[t2][:],
                                     start=True, stop=False)
                    psB = pbp.tile([128, TT], FP32, tag="ps", name=f"psB_{t2}")
                    nc.tensor.matmul(psB[:], wsr_s[:, 1, :], g0s[t2][:],
                                     start=True, stop=True)
                    nc.vector.scalar_tensor_tensor(
                        z1_s[:, 2 + t0:2 + t0 + TT], psB[:], bo0_v,
                        z0_s[:, 1 + t0:1 + t0 + TT], ALU.add, ALU.add)

                emit_out_chunk(t3, 0)

                # -- block 1 of t2: g1 = gate(conv(z1, wd1, dil=2))
                if 0 <= t2 < NT:
                    t0 = TT * t2
                    pg1 = pbp.tile([128, TT], FP32, tag="ps", name=f"pg1_{t2}")
                    nc.tensor.matmul(pg1[:], wd_s[:, 2, :],
                                     z1_s[:, t0:t0 + TT],
                                     start=True, stop=False)
                    nc.tensor.matmul(pg1[:], wd_s[:, 3, :],
                                     z1_s[:, 2 + t0:2 + t0 + TT],
                                     start=False, stop=True)
                    a1 = actp.tile([128, TT], BF16, tag="a", name=f"a1_{t2}")
                    nc.scalar.activation(a1[:], pg1[:], AF.Tanh, bias=bd_v[1])
                    b1 = actp.tile([128, TT], BF16, tag="b", name=f"b1_{t2}")
                    nc.scalar.activation(b1[:], pg1[:], AF.Sigmoid,
                                         bias=bd_v[1])
                    g1s[t2] = gtp.tile([128, TT], BF16, tag="g1",
                                       name=f"g1_{t2}")
                    nc.gpsimd.tensor_mul(g1s[t2][:], a1[:], b1[:])

                emit_out_chunk(t3, 1)

                # -- block 0 of t1: g0 = gate(conv(z0, wd0, dil=1))
                if 0 <= t1 < NT:
                    t0 = TT * t1
                    pg0 = pbp.tile([128, TT], FP32, tag="ps", name=f"pg0_{t1}")
                    filler(pg0[:], wd_s[:, 0, :])
                    nc.tensor.matmul(pg0[:], wd_s[:, 0, :],
                                     z0_s[:, t0:t0 + TT],
                                     start=True, stop=False)
                    nc.tensor.matmul(pg0[:], wd_s[:, 1, :],
                                     z0_s[:, 1 + t0:1 + t0 + TT],
                                     start=False, stop=True)
                    a0 = actp.tile([128, TT], BF16, tag="a", name=f"a0_{t1}")
                    nc.scalar.activation(a0[:], pg0[:], AF.Tanh, bias=bd_v[0])
                    b0 = actp.tile([128, TT], BF16, tag="b", name=f"b0_{t1}")
                    nc.scalar.activation(b0[:], pg0[:], AF.Sigmoid,
                                         bias=bd_v[0])
                    g0s[t1] = gtp.tile([128, TT], BF16, tag="g0",
                                       name=f"g0_{t1}")
                    nc.gpsimd.tensor_mul(g0s[t1][:], a0[:], b0[:])

                emit_out_chunk(t3, 2)

                # -- head of t2: close the skip accumulation, s0
                if 0 <= t2 < NT:
                    psA = psAs[t2]
                    nc.tensor.matmul(psA[:], ws1_s[:], g1s[t2][:],
                                     start=False, stop=True,
                                     skip_group_check=True)
                    s0s[t2] = gtp.tile([128, TT], BF16, tag="s0",
                                       name=f"s0_{t2}")
                    nc.vector.tensor_scalar(s0s[t2][:], psA[:], bskip_v, 0.0,
                                            ALU.add, ALU.max)

                # -- s1 pairs of t2: relu(wsk1@s0 + bias)
                if 0 <= t2 < NT:
                    c0 = t2 * NC4
                    for q in range(2):
                        ps5 = pbp.tile([97, NC4, 4], FP32, tag="ps",
                                       name=f"ps5_{t2}_{q}")
                        nc.tensor.matmul(ps5[:], wsk1_s[:, q, :], s0s[t2][:],
                                         start=True, stop=True)
                        nc.scalar.activation(s1_s[q][:, c0:c0 + NC4, :],
                                             ps5[:], AF.Relu,
                                             bias=bsk1_v[0:97])

                # pre-allocate next phase's causal PSUM with a filler so
                # the PE stays busy across the j3 group and phase boundary
                if 0 <= t1 + 1 < NT:
                    pz_next = pbp.tile([128, TT], FP32, tag="ps",
                                       name=f"pz_{t1 + 1}")
                    filler(pz_next[:], wd_s[:, 0, :])
                    pzs[t1 + 1] = pz_next

                emit_out_chunk(t3, 3, dmas=True)

                if 0 <= t1 < NT:
                    hbx = pbp.tile([128, TT], FP32, tag="ps",
                                   name=f"hbx_{t1}")
                    filler(hbx[:], wd_s[:, 0, :])

                # tail compression: fold most of the last tile's final
                # conv into the second-to-last phase
                if t1 == NT:
                    emit_out_chunk(NT - 1, 0)
                    emit_out_chunk(NT - 1, 1)

            for t in range(NT + 1):
                emit_phase(t)
            emit_out_chunk(NT - 1, 2)
            emit_out_chunk(NT - 1, 3, dmas=True)

    nc.compile()
    return nc


def get_nc():
    global _cached_nc
    if _cached_nc is None:
        _cached_nc = build_nc()
    return _cached_nc


def kernel(**inputs):
    nc = get_nc()
    w = prepare_weights(
        inputs["w_causal"], inputs["b_causal"],
        inputs["wd0"], inputs["bd0"], inputs["ws0"], inputs["bs0"],
        inputs["wo0"], inputs["bo0"],
        inputs["wd1"], inputs["bd1"], inputs["ws1"], inputs["bs1"],
        inputs["wo1"], inputs["bo1"],
        inputs["w_sk1"], inputs["b_sk1"], inputs["w_sk2"], inputs["b_sk2"])
    x = np.asarray(inputs["x"])
    in_maps = [{"xT": prepare_x(x, c), **w} for c in range(N_CORES)]
    res = run_bass_kernel_spmd(nc, in_maps, list(range(N_CORES)))
    out = np.concatenate(
        [np.asarray(res.results[c]["y"]).reshape(BPC, T, C_OUT)
         for c in range(N_CORES)], axis=0)
    return out.astype(np.float32)


# revision 37
# speedup vs baseline: 1.2642x; 1.2642x over previous
"""Trainium2 Bass kernel for the DiCNN (WaveNet-like) module.

Sharding: pure data parallelism - 4 batch items per core on 8 cores.
On-chip layout: channels on partitions, time on the free dim; the four
batch items are stacked as 4x32-partition bands (block-diag weights),
2x64 bands for the 64-channel causal layer.

v2 design notes (vs the 151us baseline):
- y is stored bf16 (halves the dominant output-store DMA stream) and
  upcast to fp32 host-side. rel tolerance is 2e-2; bf16 adds ~4e-3.
- skip path accumulates in PSUM: ws0@g0 (start) and ws1@g1 (stop)
  share one bank, so the skip copy + one STT disappear.
- s1 is kept in pair layout [97, T]: bands 2q at partitions 0:33 and
  2q+1 at 64:97 (ones rows at 32/96 fold the w_sk2 bias). One relu op
  materializes both bands, and the final-conv stationaries alternate
  PE row strips 0/64 so the HW pulls LDWEIGHTS ahead of in-flight
  matmuls (no row-group conflict) - hiding 16 weight loads per tile.
- final-conv time chunks are stride-4 interleaved (chunk j covers
  t = t0+4p+j), so the 4 copied chunks land t-major in SBUF and one
  DMA per (band, tile) stores 512 timesteps contiguously.
- gate multiplies run on GpSimd (only SBUF-resident op available to
  it; PSUM is V/S-only), PSUM->SBUF out-copies split scalar/vector.
- causal conv uses [128, 64] col-split stationaries (cheaper LDW).
"""

import numpy as np
import ml_dtypes

import concourse.bacc as bacc
import concourse.tile as tile
from concourse import mybir
from concourse.bass_utils import run_bass_kernel_spmd

BF16 = mybir.dt.bfloat16
FP32 = mybir.dt.float32

B, T, C_IN, HID, C_OUT, K = 32, 4096, 64, 32, 448, 2
N_CORES = 8
BPC = B // N_CORES          # batches per core = 4
TT = 512                    # time-tile size
NT = T // TT                # 8 tiles
NC4 = TT // 4               # 128 columns per interleaved chunk
XROWS = 4112                # 4097 rounded up (row 0 = causal zero pad)
XSPLIT = 2564               # x loads split so tiles 0-4 unblock early
N_WARMUP = 14               # dependency-free warm-up matmuls

AF = mybir.ActivationFunctionType
ALU = mybir.AluOpType

# which of the 16 out-copies per tile run on the scalar engine
# (indices 4*j+b; spread through the phase so pout slots recycle
# steadily and neither engine's queue bunches up)
SCALAR_COPY = {1, 3, 4, 6, 12, 14}

_cached_nc = None


def _f(x):
    return np.asarray(x, dtype=np.float32)


def _bf(x):
    return np.asarray(x, dtype=np.float32).astype(ml_dtypes.bfloat16)


def _tile4(v):
    return np.tile(_f(v).reshape(-1), 4)


def prepare_weights(w_causal, b_causal, wd0, bd0, ws0, bs0, wo0, bo0,
                    wd1, bd1, ws1, bs1, wo1, bo1, w_sk1, b_sk1, w_sk2, b_sk2):
    """Host-side weight layout transforms (identical for every core)."""
    del wo1, bo1  # dead code: z after the last block is never used

    def diag4(w32):
        s = np.zeros((128, 128), np.float32)
        for i in range(4):
            s[32 * i:32 * i + 32, 32 * i:32 * i + 32] = w32
        return s

    wc = np.zeros((128, 4, 128), np.float32)
    for p in range(2):
        for k in range(2):
            wcT = _f(w_causal)[:, :, k].T
            s = np.zeros((128, 128), np.float32)
            s[0:64, 64 * p:64 * p + 32] = wcT
            s[64:128, 64 * p + 32:64 * p + 64] = wcT
            wc[:, 2 * p + k, :] = s

    wd = np.zeros((128, 4, 128), np.float32)
    for blk, w in enumerate((wd0, wd1)):
        for k in range(2):
            wd[:, 2 * blk + k, :] = diag4(_f(w)[:, :, k].T)

    wsr = np.zeros((128, 2, 128), np.float32)
    wsr[:, 0, :] = diag4(_f(ws0)[:, :, 0].T)
    wsr[:, 1, :] = diag4(_f(wo0)[:, :, 0].T)
    ws1d = diag4(_f(ws1)[:, :, 0].T)

    # w_sk1 pair stationaries: wsk1[:, q, :] covers batch bands 2q, 2q+1;
    # output cols 32 and 96 stay 0 so relu(0 + 1.0 bias) makes ones rows.
    wsk1 = np.zeros((128, 2, 97), np.float32)
    w1T = _f(w_sk1)[:, :, 0].T
    for q in range(2):
        wsk1[64 * q:64 * q + 32, q, 0:32] = w1T
        wsk1[64 * q + 32:64 * q + 64, q, 64:96] = w1T

    # final conv moving operand, duplicated at partition bases 0 and 64
    w2d = np.zeros((97, 448), np.float32)
    for base in (0, 64):
        w2d[base:base + 32, :] = _f(w_sk2)[:, :, 0].T
        w2d[base + 32, :] = _f(b_sk2)

    bvecs = np.zeros((128, 6), np.float32)
    bvecs[:, 0] = _tile4(b_causal)
    bvecs[:, 1] = _tile4(bd0)
    bvecs[:, 2] = _tile4(bd1)
    bvecs[:, 3] = _tile4(bo0)
    bvecs[:, 4] = _tile4(_f(bs0) + _f(bs1))
    bvecs[0:32, 5] = _f(b_sk1)
    bvecs[32, 5] = 1.0
    bvecs[64:96, 5] = _f(b_sk1)
    bvecs[96, 5] = 1.0

    return dict(
        wc=_bf(wc), wd=_bf(wd), wsr=_bf(wsr), ws1d=_bf(ws1d),
        wsk1=_bf(wsk1), w2d=_bf(w2d), bvecs=np.ascontiguousarray(bvecs),
    )


def prepare_x(x, core):
    """Per-core channel-major input staging array [2, 128, XROWS] bf16.

    Column 0 is the causal zero pad (t=-1); column 1+t holds x[b, t, :]
    for the two batches of pair p stacked on the partition axis.
    """
    xT = np.zeros((2, 128, XROWS), ml_dtypes.bfloat16)
    xb = _bf(x)
    for p in range(2):
        xT[p, 0:64, 1:1 + T] = xb[4 * core + 2 * p].T
        xT[p, 64:128, 1:1 + T] = xb[4 * core + 2 * p + 1].T
    return xT


def build_nc():
    nc = bacc.Bacc("TRN2", target_bir_lowering=False, debug=False,
                   num_devices=N_CORES)

    xT_d = nc.dram_tensor("xT", [2, 128, XROWS], BF16, kind="ExternalInput")
    wc_d = nc.dram_tensor("wc", [128, 4, 128], BF16, kind="ExternalInput")
    wd_d = nc.dram_tensor("wd", [128, 4, 128], BF16, kind="ExternalInput")
    wsr_d = nc.dram_tensor("wsr", [128, 2, 128], BF16, kind="ExternalInput")
    ws1_d = nc.dram_tensor("ws1d", [128, 128], BF16, kind="ExternalInput")
    wsk1_d = nc.dram_tensor("wsk1", [128, 2, 97], BF16, kind="ExternalInput")
    w2_d = nc.dram_tensor("w2d", [97, 448], BF16, kind="ExternalInput")
    bv_d = nc.dram_tensor("bvecs", [128, 6], FP32, kind="ExternalInput")
    # [b, it, p, j, co] has the same linear layout as [b, t, co] with
    # t = 512*it + 4*p + j  (the stride-4 chunk interleave).
    y_d = nc.dram_tensor("y", [BPC, NT, NC4, 4, C_OUT], BF16,
                         kind="ExternalOutput")

    with tile.TileContext(nc) as tc:
        with (
            tc.tile_pool(name="const", bufs=1) as const,
            tc.tile_pool(name="persist", bufs=1) as persist,
            tc.tile_pool(name="act", bufs=4) as actp,
            tc.tile_pool(name="gtile", bufs=6) as gtp,
            tc.tile_pool(name="outsb", bufs=6) as outsbp,
            tc.tile_pool(name="pbody", bufs=4, space="PSUM") as pbp,
            tc.tile_pool(name="pout", bufs=4, space="PSUM") as poutp,
        ):
            # ---- load order matters: the sync HWDGE ring drains FIFO.
            # Tiny wc+bvecs first (warm-up + ACT preload), then the early
            # x halves, then the other weights, then the late x halves.
            x_s = [persist.tile([128, XROWS], BF16, tag=f"x{p}", name=f"x_s{p}")
                   for p in range(2)]
            wc_s = const.tile([128, 4, 128], BF16)
            nc.sync.dma_start(wc_s[:], wc_d.ap())
            bv_s = const.tile([128, 6], FP32)
            nc.sync.dma_start(bv_s[:], bv_d.ap())
            for p in range(2):
                nc.sync.dma_start(x_s[p][:, 0:XSPLIT], xT_d[p][:, 0:XSPLIT])
            wd_s = const.tile([128, 4, 128], BF16)
            nc.sync.dma_start(wd_s[:], wd_d.ap())
            wsr_s = const.tile([128, 2, 128], BF16)
            nc.sync.dma_start(wsr_s[:], wsr_d.ap())
            ws1_s = const.tile([128, 128], BF16)
            nc.sync.dma_start(ws1_s[:], ws1_d.ap())
            wsk1_s = const.tile([128, 2, 97], BF16)
            nc.sync.dma_start(wsk1_s[:], wsk1_d.ap())
            w2_s = const.tile([97, 448], BF16)
            nc.sync.dma_start(w2_s[:], w2_d.ap())
            for p in range(2):
                nc.sync.dma_start(x_s[p][:, XSPLIT:XROWS],
                                  xT_d[p][:, XSPLIT:XROWS])

            # preload every ACT function table set during the x transfer
            # (the first real activation would otherwise stall ~1.3us)
            scratch = const.tile([128, 8], FP32)
            nc.scalar.activation(scratch[:, 0:1], bv_s[:, 0:1], AF.Tanh)
            nc.scalar.activation(scratch[:, 1:2], bv_s[:, 0:1], AF.Sigmoid)
            nc.scalar.activation(scratch[:, 2:3], bv_s[:, 0:1], AF.Relu)
            nc.scalar.add(scratch[:, 3:4], bv_s[:, 0:1], 0.0)
            nc.scalar.copy(scratch[:, 4:5], bv_s[:, 0:1])

            bcausal = bv_s[:, 0:1]
            bd_v = (bv_s[:, 1:2], bv_s[:, 2:3])
            bo0_v = bv_s[:, 3:4]
            bskip_v = bv_s[:, 4:5]
            bsk1_v = bv_s[:, 5:6]

            # ---- persistent activations ----
            z0_s = persist.tile([128, 4100], BF16, tag="z0")
            nc.vector.memset(z0_s[:, 0:1], 0.0)
            z1_s = persist.tile([128, 4100], BF16, tag="z1")
            nc.vector.memset(z1_s[:, 0:2], 0.0)
            # s1 pair buffers: [97, T/4, 4]; band 2q at partitions 0:33,
            # band 2q+1 at 64:97 (ones rows at 32 / 96).
            s1_s = [persist.tile([97, T // 4, 4], BF16, tag=f"s1_{q}",
                                 name=f"s1_{q}") for q in range(2)]

            # ---- PE warm-up burst (overlaps the x loads) ----
            wu_t = persist.tile([128, TT], BF16, tag="wu")
            nc.vector.memset(wu_t[:], 0.0)
            hb_cnt = [0]

            def heartbeat(n):
                """Dependency-free PE filler matmuls: keep the HAM activity
                window busy across dependency stalls so the 2.4 GHz clock
                state is never lost."""
                for _ in range(n):
                    pwu = poutp.tile([128, C_OUT], FP32, tag="po",
                                     name=f"pwu_{hb_cnt[0]}")
                    hb_cnt[0] += 1
                    nc.tensor.matmul(pwu[:], wc_s[:, 0, :], wu_t[:, 0:C_OUT],
                                     start=True, stop=True)

            heartbeat(N_WARMUP)

            def filler(tile_ap, lhsT):
                """Garbage matmul into a pool slot that is free at this
                point; the real producer overwrites it with start=True.
                Pads PE idle gaps so the HAM activity window stays busy."""
                hb_cnt[0] += 1
                nc.tensor.matmul(tile_ap, lhsT, wu_t[:],
                                 start=True, stop=True)

            outsb_tiles = {}
            g0s, g1s, s0s, psAs, pzs = {}, {}, {}, {}, {}

            def outsb_of(it, b):
                key = (it, b)
                if key not in outsb_tiles:
                    outsb_tiles[key] = outsbp.tile(
                        [128, 4, C_OUT], BF16, tag="o", name=f"o_{it}_{b}")
                return outsb_tiles[key]

            def emit_out_chunk(it, j, dmas=False):
                """Final-conv work for (tile it, interleaved chunk j):
                4 matmuls (row strips alternate 0/64 so LDW pulls ahead)
                + 4 PSUM->SBUF bf16 copies. Chunk j covers t = t0+4p+j."""
                if it < 0:
                    heartbeat(3)
                    return
                c0 = it * NC4
                for b in range(4):
                    q, hi = b // 2, b % 2
                    base = 64 * hi
                    po = poutp.tile([128, C_OUT], FP32, tag="po",
                                    name=f"po_{it}_{j}_{b}")
                    nc.tensor.matmul(
                        po[:], s1_s[q][base:base + 33, c0:c0 + NC4, j],
                        w2_s[base:base + 33, :], start=True, stop=True)
                    o_t = outsb_of(it, b)
                    if (4 * j + b) in SCALAR_COPY:
                        nc.scalar.copy(o_t[:, j, :], po[:])
                    else:
                        nc.vector.tensor_copy(o_t[:, j, :], po[:])
                if dmas:
                    for b in range(4):
                        nc.sync.dma_start(y_d[b, it], outsb_of(it, b)[:])

            def emit_phase(t):
                """Software-pipelined phase: stage1 of tile t, stage2+3 of
                t-1, final conv + store of t-2. All cross-stage producers
                complete >=1 phase (or several us) before their consumers,
                so no engine queue blocks long."""
                t1, t2, t3 = t, t - 1, t - 2

                # -- causal conv(t1): 2 batch-pair groups of 2 taps -> z0
                if 0 <= t1 < NT:
                    t0 = TT * t1
                    pz = pzs.pop(t1, None)
                    if pz is None:
                        pz = pbp.tile([128, TT], FP32, tag="ps",
                                      name=f"pz_{t1}")
                    for p in range(2):
                        rhs = (x_s[p][:, t0:t0 + TT],
                               x_s[p][:, t0 + 1:t0 + 1 + TT])
                        for k in range(2):
                            nc.tensor.matmul(
                                pz[64 * p:64 * p + 64, :],
                                wc_s[:, 2 * p + k, 64 * p:64 * p + 64], rhs[k],
                                start=(k == 0), stop=(k == 1))
                    nc.scalar.add(z0_s[:, 1 + t0:1 + t0 + TT], pz[:], bcausal)

                # -- skip0/res0 matmuls (t2): consume g0 from last phase
                if 0 <= t2 < NT:
                    t0 = TT * t2
                    psA = pbp.tile([128, TT], FP32, tag="ps",
                                   name=f"psA_{t2}")
                    psAs[t2] = psA
                    nc.tensor.matmul(psA[:], wsr_s[:, 0, :], g0s[t2][:],
                                     start=True, stop=False)
                    psB = pbp.tile([128, TT], FP32, tag="ps", name=f"psB_{t2}")
                    nc.tensor.matmul(psB[:], wsr_s[:, 1, :], g0s[t2][:],
                                     start=True, stop=True)
                    nc.vector.scalar_tensor_tensor(
                        z1_s[:, 2 + t0:2 + t0 + TT], psB[:], bo0_v,
                        z0_s[:, 1 + t0:1 + t0 + TT], ALU.add, ALU.add)

                emit_out_chunk(t3, 0)

                # -- block 1 of t2: g1 = gate(conv(z1, wd1, dil=2))
                if 0 <= t2 < NT:
                    t0 = TT * t2
                    pg1 = pbp.tile([128, TT], FP32, tag="ps", name=f"pg1_{t2}")
                    nc.tensor.matmul(pg1[:], wd_s[:, 2, :],
                                     z1_s[:, t0:t0 + TT],
                                     start=True, stop=False)
                    nc.tensor.matmul(pg1[:], wd_s[:, 3, :],
                                     z1_s[:, 2 + t0:2 + t0 + TT],
                                     start=False, stop=True)
                    a1 = actp.tile([128, TT], BF16, tag="a", name=f"a1_{t2}")
                    nc.scalar.activation(a1[:], pg1[:], AF.Tanh, bias=bd_v[1])
                    b1 = actp.tile([128, TT], BF16, tag="b", name=f"b1_{t2}")
                    nc.scalar.activation(b1[:], pg1[:], AF.Sigmoid,
                                         bias=bd_v[1])
                    g1s[t2] = gtp.tile([128, TT], BF16, tag="g1",
                                       name=f"g1_{t2}")
                    nc.vector.tensor_mul(g1s[t2][:], a1[:], b1[:])

                emit_out_chunk(t3, 1)

                # -- block 0 of t1: g0 = gate(conv(z0, wd0, dil=1))
                if 0 <= t1 < NT:
                    t0 = TT * t1
                    pg0 = pbp.tile([128, TT], FP32, tag="ps", name=f"pg0_{t1}")
                    filler(pg0[:], wd_s[:, 0, :])
                    nc.tensor.matmul(pg0[:], wd_s[:, 0, :],
                                     z0_s[:, t0:t0 + TT],
                                     start=True, stop=False)
                    nc.tensor.matmul(pg0[:], wd_s[:, 1, :],
                                     z0_s[:, 1 + t0:1 + t0 + TT],
                                     start=False, stop=True)
                    a0 = actp.tile([128, TT], BF16, tag="a", name=f"a0_{t1}")
                    nc.scalar.activation(a0[:], pg0[:], AF.Tanh, bias=bd_v[0])
                    b0 = actp.tile([128, TT], BF16, tag="b", name=f"b0_{t1}")
                    nc.scalar.activation(b0[:], pg0[:], AF.Sigmoid,
                                         bias=bd_v[0])
                    g0s[t1] = gtp.tile([128, TT], BF16, tag="g0",
                                       name=f"g0_{t1}")
                    nc.gpsimd.tensor_mul(g0s[t1][:], a0[:], b0[:])

                emit_out_chunk(t3, 2)

                # -- head of t2: close the skip accumulation, s0
                if 0 <= t2 < NT:
                    psA = psAs[t2]
                    nc.tensor.matmul(psA[:], ws1_s[:], g1s[t2][:],
                                     start=False, stop=True,
                                     skip_group_check=True)
                    s0s[t2] = gtp.tile([128, TT], BF16, tag="s0",
                                       name=f"s0_{t2}")
                    nc.vector.tensor_scalar(s0s[t2][:], psA[:], bskip_v, 0.0,
                                            ALU.add, ALU.max)

                # -- s1 pairs of t2: relu(wsk1@s0 + bias)
                if 0 <= t2 < NT:
                    c0 = t2 * NC4
                    for q in range(2):
                        ps5 = pbp.tile([97, NC4, 4], FP32, tag="ps",
                                       name=f"ps5_{t2}_{q}")
                        nc.tensor.matmul(ps5[:], wsk1_s[:, q, :], s0s[t2][:],
                                         start=True, stop=True)
                        nc.scalar.activation(s1_s[q][:, c0:c0 + NC4, :],
                                             ps5[:], AF.Relu,
                                             bias=bsk1_v[0:97])

                # pre-allocate next phase's causal PSUM with a filler so
                # the PE stays busy across the j3 group and phase boundary
                if 0 <= t1 + 1 < NT:
                    pz_next = pbp.tile([128, TT], FP32, tag="ps",
                                       name=f"pz_{t1 + 1}")
                    filler(pz_next[:], wd_s[:, 0, :])
                    pzs[t1 + 1] = pz_next

                emit_out_chunk(t3, 3, dmas=True)

                if 0 <= t1 < NT:
                    hbx = pbp.tile([128, TT], FP32, tag="ps",
                                   name=f"hbx_{t1}")
                    filler(hbx[:], wd_s[:, 0, :])

                # tail compression: fold most of the last tile's final
                # conv into the second-to-last phase
                if t1 == NT:
                    emit_out_chunk(NT - 1, 0)
                    emit_out_chunk(NT - 1, 1)

            for t in range(NT + 1):
                emit_phase(t)
            emit_out_chunk(NT - 1, 2)
            emit_out_chunk(NT - 1, 3, dmas=True)

    nc.compile()
    return nc


def get_nc():
    global _cached_nc
    if _cached_nc is None:
        _cached_nc = build_nc()
    return _cached_nc


def kernel(**inputs):
    nc = get_nc()
    w = prepare_weights(
        inputs["w_causal"], inputs["b_causal"],
        inputs["wd0"], inputs["bd0"], inputs["ws0"], inputs["bs0"],
        inputs["wo0"], inputs["bo0"],
        inputs["wd1"], inputs["bd1"], inputs["ws1"], inputs["bs1"],
        inputs["wo1"], inputs["bo1"],
        inputs["w_sk1"], inputs["b_sk1"], inputs["w_sk2"], inputs["b_sk2"])
    x = np.asarray(inputs["x"])
    in_maps = [{"xT": prepare_x(x, c), **w} for c in range(N_CORES)]
    res = run_bass_kernel_spmd(nc, in_maps, list(range(N_CORES)))
    out = np.concatenate(
        [np.asarray(res.results[c]["y"]).reshape(BPC, T, C_OUT)
         for c in range(N_CORES)], axis=0)
    return out.astype(np.float32)


# revision 40
# speedup vs baseline: 1.3083x; 1.0349x over previous
"""Trainium2 Bass kernel for the DiCNN (WaveNet-like) module.

Sharding: pure data parallelism - 4 batch items per core on 8 cores.
On-chip layout: channels on partitions, time on the free dim; the four
batch items are stacked as 4x32-partition bands (block-diag weights),
2x64 bands for the 64-channel causal layer.

v2 design notes (vs the 151us baseline):
- y is stored bf16 (halves the dominant output-store DMA stream) and
  upcast to fp32 host-side. rel tolerance is 2e-2; bf16 adds ~4e-3.
- skip path accumulates in PSUM: ws0@g0 (start) and ws1@g1 (stop)
  share one bank, so the skip copy + one STT disappear.
- s1 is kept in pair layout [97, T]: bands 2q at partitions 0:33 and
  2q+1 at 64:97 (ones rows at 32/96 fold the w_sk2 bias). One relu op
  materializes both bands, and the final-conv stationaries alternate
  PE row strips 0/64 so the HW pulls LDWEIGHTS ahead of in-flight
  matmuls (no row-group conflict) - hiding 16 weight loads per tile.
- final-conv time chunks are stride-4 interleaved (chunk j covers
  t = t0+4p+j), so the 4 copied chunks land t-major in SBUF and one
  DMA per (band, tile) stores 512 timesteps contiguously.
- gate multiplies run on GpSimd (only SBUF-resident op available to
  it; PSUM is V/S-only), PSUM->SBUF out-copies split scalar/vector.
- causal conv uses [128, 64] col-split stationaries (cheaper LDW).
"""

import numpy as np
import ml_dtypes

import concourse.bacc as bacc
import concourse.tile as tile
from concourse import mybir
from concourse.bass_utils import run_bass_kernel_spmd

BF16 = mybir.dt.bfloat16
FP32 = mybir.dt.float32

B, T, C_IN, HID, C_OUT, K = 32, 4096, 64, 32, 448, 2
N_CORES = 8
BPC = B // N_CORES          # batches per core = 4
TT = 512                    # time-tile size
NT = T // TT                # 8 tiles
NC4 = TT // 4               # 128 columns per interleaved chunk
XROWS = 4112                # 4097 rounded up (row 0 = causal zero pad)
XSPLIT = 2564               # x loads split so tiles 0-4 unblock early
N_WARMUP = 14               # dependency-free warm-up matmuls

AF = mybir.ActivationFunctionType
ALU = mybir.AluOpType

# which of the 16 out-copies per tile run on the scalar engine
# (indices 4*j+b; spread through the phase so pout slots recycle
# steadily and neither engine's queue bunches up)
SCALAR_COPY = {1, 3, 4, 6, 9, 14}

_cached_nc = None


def _f(x):
    return np.asarray(x, dtype=np.float32)


def _bf(x):
    return np.asarray(x, dtype=np.float32).astype(ml_dtypes.bfloat16)


def _tile4(v):
    return np.tile(_f(v).reshape(-1), 4)


def prepare_weights(w_causal, b_causal, wd0, bd0, ws0, bs0, wo0, bo0,
                    wd1, bd1, ws1, bs1, wo1, bo1, w_sk1, b_sk1, w_sk2, b_sk2):
    """Host-side weight layout transforms (identical for every core)."""
    del wo1, bo1  # dead code: z after the last block is never used

    def diag4(w32):
        s = np.zeros((128, 128), np.float32)
        for i in range(4):
            s[32 * i:32 * i + 32, 32 * i:32 * i + 32] = w32
        return s

    wc = np.zeros((128, 4, 128), np.float32)
    for p in range(2):
        for k in range(2):
            wcT = _f(w_causal)[:, :, k].T
            s = np.zeros((128, 128), np.float32)
            s[0:64, 64 * p:64 * p + 32] = wcT
            s[64:128, 64 * p + 32:64 * p + 64] = wcT
            wc[:, 2 * p + k, :] = s

    wd = np.zeros((128, 4, 128), np.float32)
    for blk, w in enumerate((wd0, wd1)):
        for k in range(2):
            wd[:, 2 * blk + k, :] = diag4(_f(w)[:, :, k].T)

    wsr = np.zeros((128, 2, 128), np.float32)
    wsr[:, 0, :] = diag4(_f(ws0)[:, :, 0].T)
    wsr[:, 1, :] = diag4(_f(wo0)[:, :, 0].T)
    ws1d = diag4(_f(ws1)[:, :, 0].T)

    # w_sk1 pair stationaries: wsk1[:, q, :] covers batch bands 2q, 2q+1;
    # output cols 32 and 96 stay 0 so relu(0 + 1.0 bias) makes ones rows.
    wsk1 = np.zeros((128, 2, 97), np.float32)
    w1T = _f(w_sk1)[:, :, 0].T
    for q in range(2):
        wsk1[64 * q:64 * q + 32, q, 0:32] = w1T
        wsk1[64 * q + 32:64 * q + 64, q, 64:96] = w1T

    # final conv moving operand, duplicated at partition bases 0 and 64
    w2d = np.zeros((97, 448), np.float32)
    for base in (0, 64):
        w2d[base:base + 32, :] = _f(w_sk2)[:, :, 0].T
        w2d[base + 32, :] = _f(b_sk2)

    bvecs = np.zeros((128, 6), np.float32)
    bvecs[:, 0] = _tile4(b_causal)
    bvecs[:, 1] = _tile4(bd0)
    bvecs[:, 2] = _tile4(bd1)
    bvecs[:, 3] = _tile4(bo0)
    bvecs[:, 4] = _tile4(_f(bs0) + _f(bs1))
    bvecs[0:32, 5] = _f(b_sk1)
    bvecs[32, 5] = 1.0
    bvecs[64:96, 5] = _f(b_sk1)
    bvecs[96, 5] = 1.0

    return dict(
        wc=_bf(wc), wd=_bf(wd), wsr=_bf(wsr), ws1d=_bf(ws1d),
        wsk1=_bf(wsk1), w2d=_bf(w2d), bvecs=np.ascontiguousarray(bvecs),
    )


def prepare_x(x, core):
    """Per-core channel-major input staging array [2, 128, XROWS] bf16.

    Column 0 is the causal zero pad (t=-1); column 1+t holds x[b, t, :]
    for the two batches of pair p stacked on the partition axis.
    """
    xT = np.zeros((2, 128, XROWS), ml_dtypes.bfloat16)
    xb = _bf(x)
    for p in range(2):
        xT[p, 0:64, 1:1 + T] = xb[4 * core + 2 * p].T
        xT[p, 64:128, 1:1 + T] = xb[4 * core + 2 * p + 1].T
    return xT


def build_nc():
    nc = bacc.Bacc("TRN2", target_bir_lowering=False, debug=False,
                   num_devices=N_CORES)

    xT_d = nc.dram_tensor("xT", [2, 128, XROWS], BF16, kind="ExternalInput")
    wc_d = nc.dram_tensor("wc", [128, 4, 128], BF16, kind="ExternalInput")
    wd_d = nc.dram_tensor("wd", [128, 4, 128], BF16, kind="ExternalInput")
    wsr_d = nc.dram_tensor("wsr", [128, 2, 128], BF16, kind="ExternalInput")
    ws1_d = nc.dram_tensor("ws1d", [128, 128], BF16, kind="ExternalInput")
    wsk1_d = nc.dram_tensor("wsk1", [128, 2, 97], BF16, kind="ExternalInput")
    w2_d = nc.dram_tensor("w2d", [97, 448], BF16, kind="ExternalInput")
    bv_d = nc.dram_tensor("bvecs", [128, 6], FP32, kind="ExternalInput")
    # [b, it, p, j, co] has the same linear layout as [b, t, co] with
    # t = 512*it + 4*p + j  (the stride-4 chunk interleave).
    y_d = nc.dram_tensor("y", [BPC, NT, NC4, 4, C_OUT], BF16,
                         kind="ExternalOutput")

    with tile.TileContext(nc) as tc:
        with (
            tc.tile_pool(name="const", bufs=1) as const,
            tc.tile_pool(name="persist", bufs=1) as persist,
            tc.tile_pool(name="act", bufs=4) as actp,
            tc.tile_pool(name="gtile", bufs=6) as gtp,
            tc.tile_pool(name="outsb", bufs=6) as outsbp,
            tc.tile_pool(name="pbody", bufs=4, space="PSUM") as pbp,
            tc.tile_pool(name="pout", bufs=4, space="PSUM") as poutp,
        ):
            # ---- load order matters: the sync HWDGE ring drains FIFO.
            # Tiny wc+bvecs first (warm-up + ACT preload), then the early
            # x halves, then the other weights, then the late x halves.
            x_s = [persist.tile([128, XROWS], BF16, tag=f"x{p}", name=f"x_s{p}")
                   for p in range(2)]
            wc_s = const.tile([128, 4, 128], BF16)
            nc.sync.dma_start(wc_s[:], wc_d.ap())
            bv_s = const.tile([128, 6], FP32)
            nc.sync.dma_start(bv_s[:], bv_d.ap())
            for p in range(2):
                nc.sync.dma_start(x_s[p][:, 0:XSPLIT], xT_d[p][:, 0:XSPLIT])
            wd_s = const.tile([128, 4, 128], BF16)
            nc.sync.dma_start(wd_s[:], wd_d.ap())
            wsr_s = const.tile([128, 2, 128], BF16)
            nc.sync.dma_start(wsr_s[:], wsr_d.ap())
            ws1_s = const.tile([128, 128], BF16)
            nc.sync.dma_start(ws1_s[:], ws1_d.ap())
            wsk1_s = const.tile([128, 2, 97], BF16)
            nc.sync.dma_start(wsk1_s[:], wsk1_d.ap())
            w2_s = const.tile([97, 448], BF16)
            nc.sync.dma_start(w2_s[:], w2_d.ap())
            for p in range(2):
                nc.sync.dma_start(x_s[p][:, XSPLIT:XROWS],
                                  xT_d[p][:, XSPLIT:XROWS])

            # preload every ACT function table set during the x transfer
            # (the first real activation would otherwise stall ~1.3us)
            scratch = const.tile([128, 8], FP32)
            nc.scalar.activation(scratch[:, 0:1], bv_s[:, 0:1], AF.Tanh)
            nc.scalar.activation(scratch[:, 1:2], bv_s[:, 0:1], AF.Sigmoid)
            nc.scalar.activation(scratch[:, 2:3], bv_s[:, 0:1], AF.Relu)
            nc.scalar.add(scratch[:, 3:4], bv_s[:, 0:1], 0.0)
            nc.scalar.copy(scratch[:, 4:5], bv_s[:, 0:1])

            bcausal = bv_s[:, 0:1]
            bd_v = (bv_s[:, 1:2], bv_s[:, 2:3])
            bo0_v = bv_s[:, 3:4]
            bskip_v = bv_s[:, 4:5]
            bsk1_v = bv_s[:, 5:6]

            # ---- persistent activations ----
            z0_s = persist.tile([128, 4100], BF16, tag="z0")
            nc.vector.memset(z0_s[:, 0:1], 0.0)
            z1_s = persist.tile([128, 4100], BF16, tag="z1")
            nc.vector.memset(z1_s[:, 0:2], 0.0)
            # s1 pair buffers: [97, T/4, 4]; band 2q at partitions 0:33,
            # band 2q+1 at 64:97 (ones rows at 32 / 96).
            s1_s = [persist.tile([97, T // 4, 4], BF16, tag=f"s1_{q}",
                                 name=f"s1_{q}") for q in range(2)]

            # ---- PE warm-up burst: stationary comes from a memset (no
            # DMA dependency) so it starts during the framework preamble
            wu_t = persist.tile([128, TT], BF16, tag="wu")
            nc.vector.memset(wu_t[:], 0.0)
            wu_w = persist.tile([128, 128], BF16, tag="wuw")
            nc.vector.memset(wu_w[:], 0.0)
            hb_cnt = [0]

            def heartbeat(n):
                """Dependency-free PE filler matmuls: keep the HAM activity
                window busy across dependency stalls so the 2.4 GHz clock
                state is never lost."""
                for _ in range(n):
                    pwu = poutp.tile([128, C_OUT], FP32, tag="po",
                                     name=f"pwu_{hb_cnt[0]}")
                    hb_cnt[0] += 1
                    nc.tensor.matmul(pwu[:], wu_w[:], wu_t[:, 0:C_OUT],
                                     start=True, stop=True)

            heartbeat(N_WARMUP)

            def filler(tile_ap, lhsT):
                """Garbage matmul into a pool slot that is free at this
                point; the real producer overwrites it with start=True.
                Pads PE idle gaps so the HAM activity window stays busy."""
                hb_cnt[0] += 1
                nc.tensor.matmul(tile_ap, lhsT, wu_t[:],
                                 start=True, stop=True)

            outsb_tiles = {}
            g0s, g1s, s0s, psAs, pzs = {}, {}, {}, {}, {}

            def outsb_of(it, b):
                key = (it, b)
                if key not in outsb_tiles:
                    outsb_tiles[key] = outsbp.tile(
                        [128, 4, C_OUT], BF16, tag="o", name=f"o_{it}_{b}")
                return outsb_tiles[key]

            def emit_out_chunk(it, j, dmas=False):
                """Final-conv work for (tile it, interleaved chunk j):
                4 matmuls (row strips alternate 0/64 so LDW pulls ahead)
                + 4 PSUM->SBUF bf16 copies. Chunk j covers t = t0+4p+j."""
                if it < 0:
                    heartbeat(3)
                    return
                c0 = it * NC4
                for b in range(4):
                    q, hi = b // 2, b % 2
                    base = 64 * hi
                    po = poutp.tile([128, C_OUT], FP32, tag="po",
                                    name=f"po_{it}_{j}_{b}")
                    nc.tensor.matmul(
                        po[:], s1_s[q][base:base + 33, c0:c0 + NC4, j],
                        w2_s[base:base + 33, :], start=True, stop=True)
                    o_t = outsb_of(it, b)
                    if (4 * j + b) in SCALAR_COPY:
                        nc.scalar.copy(o_t[:, j, :], po[:])
                    else:
                        nc.vector.tensor_copy(o_t[:, j, :], po[:])
                if dmas:
                    for b in range(4):
                        nc.sync.dma_start(y_d[b, it], outsb_of(it, b)[:])

            def emit_phase(t):
                """Software-pipelined phase: stage1 of tile t, stage2+3 of
                t-1, final conv + store of t-2. All cross-stage producers
                complete >=1 phase (or several us) before their consumers,
                so no engine queue blocks long."""
                t1, t2, t3 = t, t - 1, t - 2

                # -- causal conv(t1): 2 batch-pair groups of 2 taps -> z0
                if 0 <= t1 < NT:
                    t0 = TT * t1
                    pz = pzs.pop(t1, None)
                    if pz is None:
                        pz = pbp.tile([128, TT], FP32, tag="ps",
                                      name=f"pz_{t1}")
                    for p in range(2):
                        rhs = (x_s[p][:, t0:t0 + TT],
                               x_s[p][:, t0 + 1:t0 + 1 + TT])
                        for k in range(2):
                            nc.tensor.matmul(
                                pz[64 * p:64 * p + 64, :],
                                wc_s[:, 2 * p + k, 64 * p:64 * p + 64], rhs[k],
                                start=(k == 0), stop=(k == 1))
                    nc.scalar.add(z0_s[:, 1 + t0:1 + t0 + TT], pz[:], bcausal)

                # -- skip0/res0 matmuls (t2): consume g0 from last phase
                if 0 <= t2 < NT:
                    t0 = TT * t2
                    psA = pbp.tile([128, TT], FP32, tag="ps",
                                   name=f"psA_{t2}")
                    psAs[t2] = psA
                    nc.tensor.matmul(psA[:], wsr_s[:, 0, :], g0s[t2][:],
                                     start=True, stop=False)
                    psB = pbp.tile([128, TT], FP32, tag="ps", name=f"psB_{t2}")
                    nc.tensor.matmul(psB[:], wsr_s[:, 1, :], g0s[t2][:],
                                     start=True, stop=True)
                    nc.vector.scalar_tensor_tensor(
                        z1_s[:, 2 + t0:2 + t0 + TT], psB[:], bo0_v,
                        z0_s[:, 1 + t0:1 + t0 + TT], ALU.add, ALU.add)

                emit_out_chunk(t3, 0)

                # -- block 1 of t2: g1 = gate(conv(z1, wd1, dil=2))
                if 0 <= t2 < NT:
                    t0 = TT * t2
                    pg1 = pbp.tile([128, TT], FP32, tag="ps", name=f"pg1_{t2}")
                    filler(pg1[:], wd_s[:, 2, :])
                    nc.tensor.matmul(pg1[:], wd_s[:, 2, :],
                                     z1_s[:, t0:t0 + TT],
                                     start=True, stop=False)
                    nc.tensor.matmul(pg1[:], wd_s[:, 3, :],
                                     z1_s[:, 2 + t0:2 + t0 + TT],
                                     start=False, stop=True)
                    a1 = actp.tile([128, TT], BF16, tag="a", name=f"a1_{t2}")
                    nc.scalar.activation(a1[:], pg1[:], AF.Tanh, bias=bd_v[1])
                    b1 = actp.tile([128, TT], BF16, tag="b", name=f"b1_{t2}")
                    nc.scalar.activation(b1[:], pg1[:], AF.Sigmoid,
                                         bias=bd_v[1])
                    g1s[t2] = gtp.tile([128, TT], BF16, tag="g1",
                                       name=f"g1_{t2}")
                    nc.vector.tensor_mul(g1s[t2][:], a1[:], b1[:])

                emit_out_chunk(t3, 1)

                # -- block 0 of t1: g0 = gate(conv(z0, wd0, dil=1))
                if 0 <= t1 < NT:
                    t0 = TT * t1
                    pg0 = pbp.tile([128, TT], FP32, tag="ps", name=f"pg0_{t1}")
                    filler(pg0[:], wd_s[:, 0, :])
                    nc.tensor.matmul(pg0[:], wd_s[:, 0, :],
                                     z0_s[:, t0:t0 + TT],
                                     start=True, stop=False)
                    nc.tensor.matmul(pg0[:], wd_s[:, 1, :],
                                     z0_s[:, 1 + t0:1 + t0 + TT],
                                     start=False, stop=True)
                    a0 = actp.tile([128, TT], BF16, tag="a", name=f"a0_{t1}")
                    nc.scalar.activation(a0[:], pg0[:], AF.Tanh, bias=bd_v[0])
                    b0 = actp.tile([128, TT], BF16, tag="b", name=f"b0_{t1}")
                    nc.scalar.activation(b0[:], pg0[:], AF.Sigmoid,
                                         bias=bd_v[0])
                    g0s[t1] = gtp.tile([128, TT], BF16, tag="g0",
                                       name=f"g0_{t1}")
                    nc.gpsimd.tensor_mul(g0s[t1][:], a0[:], b0[:])

                emit_out_chunk(t3, 2)

                # -- head of t2: close the skip accumulation, s0
                if 0 <= t2 < NT:
                    psA = psAs[t2]
                    nc.tensor.matmul(psA[:], ws1_s[:], g1s[t2][:],
                                     start=False, stop=True,
                                     skip_group_check=True)
                    s0s[t2] = gtp.tile([128, TT], BF16, tag="s0",
                                       name=f"s0_{t2}")
                    nc.vector.tensor_scalar(s0s[t2][:], psA[:], bskip_v, 0.0,
                                            ALU.add, ALU.max)

                # -- s1 pairs of t2: relu(wsk1@s0 + bias)
                if 0 <= t2 < NT:
                    c0 = t2 * NC4
                    for q in range(2):
                        ps5 = pbp.tile([97, NC4, 4], FP32, tag="ps",
                                       name=f"ps5_{t2}_{q}")
                        nc.tensor.matmul(ps5[:], wsk1_s[:, q, :], s0s[t2][:],
                                         start=True, stop=True)
                        nc.scalar.activation(s1_s[q][:, c0:c0 + NC4, :],
                                             ps5[:], AF.Relu,
                                             bias=bsk1_v[0:97])

                # pre-allocate next phase's causal PSUM with a filler so
                # the PE stays busy across the j3 group and phase boundary
                if 0 <= t1 + 1 < NT:
                    pz_next = pbp.tile([128, TT], FP32, tag="ps",
                                       name=f"pz_{t1 + 1}")
                    filler(pz_next[:], wd_s[:, 0, :])
                    pzs[t1 + 1] = pz_next

                emit_out_chunk(t3, 3, dmas=True)

                if 0 <= t1 < NT:
                    hbx = pbp.tile([128, TT], FP32, tag="ps",
                                   name=f"hbx_{t1}")
                    filler(hbx[:], wd_s[:, 0, :])

                # tail compression: fold most of the last tile's final
                # conv into the second-to-last phase
                if t1 == NT:
                    emit_out_chunk(NT - 1, 0)
                    emit_out_chunk(NT - 1, 1)

            for t in range(NT + 1):
                emit_phase(t)
            emit_out_chunk(NT - 1, 2)
            emit_out_chunk(NT - 1, 3, dmas=True)

    nc.compile()
    return nc


def get_nc():
    global _cached_nc
    if _cached_nc is None:
        _cached_nc = build_nc()
    return _cached_nc


def kernel(**inputs):
    nc = get_nc()
    w = prepare_weights(
        inputs["w_causal"], inputs["b_causal"],
        inputs["wd0"], inputs["bd0"], inputs["ws0"], inputs["bs0"],
        inputs["wo0"], inputs["bo0"],
        inputs["wd1"], inputs["bd1"], inputs["ws1"], inputs["bs1"],
        inputs["wo1"], inputs["bo1"],
        inputs["w_sk1"], inputs["b_sk1"], inputs["w_sk2"], inputs["b_sk2"])
    x = np.asarray(inputs["x"])
    in_maps = [{"xT": prepare_x(x, c), **w} for c in range(N_CORES)]
    res = run_bass_kernel_spmd(nc, in_maps, list(range(N_CORES)))
    out = np.concatenate(
        [np.asarray(res.results[c]["y"]).reshape(BPC, T, C_OUT)
         for c in range(N_CORES)], axis=0)
    return out.astype(np.float32)
